# revision 5
# baseline (speedup 1.0000x reference)
"""Trainium2 Bass kernel for nn_ConstraintModel (2-LSTM chain + MLP head).

Contract: kernel(**inputs) takes FULL unsharded inputs (numpy, keyed as in
setup_inputs()) and returns the FULL (512, 256, 128) float32 output.

Strategy: data-parallel over batch (256 -> 8 cores x 32). Each core runs an
identical Bass program on its batch shard:
  phase C: constraint LSTM scanned backward over the 512 steps
  phase G: generation LSTM scanned forward, consuming the stored constraint
           hiddens; per-segment MLP head; DMA out.

Layout: everything on chip is kept transposed -- [feature/hidden on SBUF
partitions, batch on the free dim] -- so the recurrent matmuls produce
gates.T directly, elementwise gate math runs on all 128 partitions, and the
new hidden state feeds the next step's matmul with no transposes anywhere.
The host pre-transposes / gate-permutes all inputs and weights.
"""

import os
import sys
from contextlib import ExitStack

sys.path.insert(0, "/opt/pypackages")
sys.path.insert(0, "/opt/trn_rl_repo")

import numpy as np
from ml_dtypes import bfloat16

import concourse.bass as bass
import concourse.bacc as bacc
import concourse.tile as tile
from concourse import mybir
from concourse.bass_utils import run_bass_kernel_spmd

F32 = mybir.dt.float32
BF16 = mybir.dt.bfloat16
AF = mybir.ActivationFunctionType
ALU = mybir.AluOpType

S_FULL = 512
B_FULL = 256
F = 128          # seq features
FC = 129         # constraint features
H = 256          # hidden (both LSTMs)
NQ = 8           # 4*H / 128 gate m-tiles
NCORES = 8
BL = B_FULL // NCORES  # 32 batch per core
TSEG = 16        # scan steps per bulk segment

# gate permutation: torch order (i, f, g, o) -> on-chip order (i, f, o, g)
# so that sigmoid covers gate m-tiles 0..5 and tanh covers 6..7.
GATE_PERM = np.r_[0:256, 256:512, 768:1024, 512:768]


# --------------------------------------------------------------------------
# host-side preparation
# --------------------------------------------------------------------------

def prep_weights(inp: dict) -> dict:
    """Gate-permute + transpose all weights. Shared across cores."""
    g = lambda w: np.ascontiguousarray(np.asarray(w, np.float32)[GATE_PERM])
    out = {}
    out["wihc"] = np.ascontiguousarray(g(inp["Wih_c"]).T)            # [129, 1024]
    out["whhc"] = np.ascontiguousarray(g(inp["Whh_c"]).T)            # [256, 1024]
    wg = g(inp["Wih_g"])                                             # [1024, 384]
    out["wgx"] = np.ascontiguousarray(wg[:, :F].T)                   # [128, 1024]
    out["wghc"] = np.ascontiguousarray(wg[:, F:].T).astype(bfloat16) # [256, 1024]
    out["whhg"] = np.ascontiguousarray(g(inp["Whh_g"]).T)            # [256, 1024]
    out["w1t"] = np.ascontiguousarray(np.asarray(inp["W1"], np.float32).T)  # [256,128]
    out["w2t"] = np.ascontiguousarray(np.asarray(inp["W2"], np.float32).T)  # [128,128]
    bc = np.asarray(inp["bih_c"], np.float32) + np.asarray(inp["bhh_c"], np.float32)
    bg = np.asarray(inp["bih_g"], np.float32) + np.asarray(inp["bhh_g"], np.float32)
    out["bc"] = np.ascontiguousarray(bc[GATE_PERM].reshape(NQ, 128).T)  # [128, 8]
    out["bg"] = np.ascontiguousarray(bg[GATE_PERM].reshape(NQ, 128).T)  # [128, 8]
    out["b1"] = np.ascontiguousarray(np.asarray(inp["b1"], np.float32)[:, None])
    out["b2"] = np.ascontiguousarray(np.asarray(inp["b2"], np.float32)[:, None])
    return out


def prep_core_inputs(seq, seq_constraints, c0, c1, s):
    """Per-core activation tensors for batch columns [c0:c1), seq len s."""
    xc = np.asarray(seq_constraints, np.float32)[:s, c0:c1]   # [s, bl, 129]
    # time-reversed + transposed: xcT[k, tau, b] = xc[s-1-tau, b, k]
    xcT = np.ascontiguousarray(xc[::-1].transpose(2, 0, 1))   # [129, s, bl]
    sq = np.asarray(seq, np.float32)[:s, c0:c1]               # [s, bl, 128]
    shifted = np.concatenate([np.zeros_like(sq[:1]), sq[:-1]], axis=0)
    xgT = np.ascontiguousarray(shifted.transpose(2, 0, 1))    # [128, s, bl]
    return {"xcT": xcT, "xgT": xgT}


# --------------------------------------------------------------------------
# device program
# --------------------------------------------------------------------------

def build_program(s=S_FULL, tseg=TSEG, bl=BL):
    """Build + compile the per-core Bass program. Returns (nc, out_name)."""
    assert s % tseg == 0
    nseg = s // tseg
    nc = bacc.Bacc("TRN2", target_bir_lowering=False, debug=False,
                   enable_asserts=False)

    d_xcT = nc.dram_tensor("xcT", [FC, s, bl], F32, kind="ExternalInput")
    d_xgT = nc.dram_tensor("xgT", [F, s, bl], F32, kind="ExternalInput")
    d_wihc = nc.dram_tensor("wihc", [FC, 4 * H], F32, kind="ExternalInput")
    d_whhc = nc.dram_tensor("whhc", [H, 4 * H], F32, kind="ExternalInput")
    d_wgx = nc.dram_tensor("wgx", [F, 4 * H], F32, kind="ExternalInput")
    d_wghc = nc.dram_tensor("wghc", [H, 4 * H], BF16, kind="ExternalInput")
    d_whhg = nc.dram_tensor("whhg", [H, 4 * H], F32, kind="ExternalInput")
    d_w1t = nc.dram_tensor("w1t", [H, F], F32, kind="ExternalInput")
    d_w2t = nc.dram_tensor("w2t", [F, F], F32, kind="ExternalInput")
    d_bc = nc.dram_tensor("bc", [128, NQ], F32, kind="ExternalInput")
    d_bg = nc.dram_tensor("bg", [128, NQ], F32, kind="ExternalInput")
    d_b1 = nc.dram_tensor("b1", [128, 1], F32, kind="ExternalInput")
    d_b2 = nc.dram_tensor("b2", [128, 1], F32, kind="ExternalInput")
    d_out = nc.dram_tensor("out", [F, s, bl], F32, kind="ExternalOutput")

    with tile.TileContext(nc) as tc, ExitStack() as ctx:
        wp = ctx.enter_context(tc.tile_pool(name="weights", bufs=1))
        hcp = ctx.enter_context(tc.tile_pool(name="hcstore", bufs=1))
        xpp = ctx.enter_context(tc.tile_pool(name="xproj", bufs=2))
        xinp = ctx.enter_context(tc.tile_pool(name="xin", bufs=2))
        hgp = ctx.enter_context(tc.tile_pool(name="hgseg", bufs=2))
        yp = ctx.enter_context(tc.tile_pool(name="yout", bufs=2))
        stp = ctx.enter_context(tc.tile_pool(name="state", bufs=3))
        ewp = ctx.enter_context(tc.tile_pool(name="eltwise", bufs=3))
        psb = ctx.enter_context(tc.tile_pool(name="psbulk", bufs=2,
                                             space=bass.MemorySpace.PSUM))
        psg = ctx.enter_context(tc.tile_pool(name="psgates", bufs=2,
                                             space=bass.MemorySpace.PSUM))

        # ---- load weights to SBUF (resident all kernel) ----
        def wtile(dram, shape, dt=F32, row0=0):
            t = wp.tile(shape, dt, tag=f"w_{dram.name}_{row0}")
            nc.sync.dma_start(t[:], dram.ap()[row0:row0 + shape[0]])
            return t

        wihc_k0 = wtile(d_wihc, [128, 4 * H])
        wihc_k1 = wtile(d_wihc, [1, 4 * H], row0=128)
        whhc = [wtile(d_whhc, [128, 4 * H], row0=128 * k) for k in range(2)]
        wgx = wtile(d_wgx, [128, 4 * H])
        wghc = [wtile(d_wghc, [128, 4 * H], BF16, row0=128 * k) for k in range(2)]
        whhg = [wtile(d_whhg, [128, 4 * H], row0=128 * k) for k in range(2)]
        w1t = [wtile(d_w1t, [128, F], row0=128 * k) for k in range(2)]
        w2t = wtile(d_w2t, [128, F])
        bc_sb = wtile(d_bc, [128, NQ])
        bg_sb = wtile(d_bg, [128, NQ])
        b1_sb = wtile(d_b1, [128, 1])
        b2_sb = wtile(d_b2, [128, 1])

        # constraint hiddens for every forward-time index t, bf16
        hc_all = hcp.tile([128, s, 2, bl], BF16, tag="hc_all")

        def scan_step(xp_tile, tl, whh, h_prev, c_prev, h_out):
            """One LSTM step. Returns (h_new_ap, c_new_tile)."""
            pg = psg.tile([128, NQ, bl], F32, tag="pg")
            for q in range(NQ):
                for r in range(4):
                    col = 128 * q + 32 * r
                    for k in range(2):
                        nc.tensor.matmul(
                            pg[32 * r:32 * r + 32, q, :],
                            whh[k][:, col:col + 32],
                            h_prev[:, k, :],
                            start=(k == 0), stop=(k == 1),
                            tile_position=(0, 32 * r),
                        )
            gs = ewp.tile([128, NQ, bl], F32, tag="gs")
            nc.vector.tensor_tensor(gs[:], pg[:], xp_tile[:, tl], ALU.add)
            sig = ewp.tile([128, 6, bl], F32, tag="sig")
            nc.scalar.activation(sig[:], gs[:, 0:6], AF.Sigmoid)
            tg = ewp.tile([128, 2, bl], F32, tag="tg")
            nc.scalar.activation(tg[:], gs[:, 6:8], AF.Tanh)
            u = ewp.tile([128, 2, bl], F32, tag="u")
            nc.vector.tensor_tensor(u[:], sig[:, 0:2], tg[:], ALU.mult)
            v = ewp.tile([128, 2, bl], F32, tag="v")
            nc.vector.tensor_tensor(v[:], sig[:, 2:4], c_prev[:], ALU.mult)
            c_new = stp.tile([128, 2, bl], F32, tag="c")
            nc.vector.tensor_tensor(c_new[:], u[:], v[:], ALU.add)
            tc2 = ewp.tile([128, 2, bl], F32, tag="tc2")
            nc.scalar.activation(tc2[:], c_new[:], AF.Tanh)
            nc.vector.tensor_tensor(h_out, sig[:, 4:6], tc2[:], ALU.mult)
            return c_new

        # =================== phase C: constraint LSTM (backward) ==========
        h_prev = stp.tile([128, 2, bl], F32, tag="h")
        c_prev = stp.tile([128, 2, bl], F32, tag="c")
        nc.vector.memset(h_prev[:], 0.0)
        nc.vector.memset(c_prev[:], 0.0)

        for seg in range(nseg):
            t0 = seg * tseg
            xc0 = xinp.tile([128, tseg, bl], F32, tag="xc0")
            nc.sync.dma_start(xc0[:], d_xcT.ap()[0:128, t0:t0 + tseg])
            xc1 = xinp.tile([1, tseg, bl], F32, tag="xc1")
            nc.sync.dma_start(xc1[:], d_xcT.ap()[128:129, t0:t0 + tseg])
            xp = xpp.tile([128, tseg, NQ, bl], F32, tag="xp")
            for q in range(NQ):
                ps = psb.tile([128, tseg, bl], F32, tag="psb")
                nc.tensor.matmul(ps[:], wihc_k0[:, 128 * q:128 * (q + 1)],
                                 xc0[:], start=True, stop=False)
                nc.tensor.matmul(ps[:], wihc_k1[:, 128 * q:128 * (q + 1)],
                                 xc1[:], start=False, stop=True)
                nc.scalar.activation(xp[:, :, q, :], ps[:], AF.Identity,
                                     bias=bc_sb[:, q:q + 1])
            for tl in range(tseg):
                t = s - 1 - (t0 + tl)           # forward-time index
                h_new = stp.tile([128, 2, bl], F32, tag="h")
                c_prev = scan_step(xp, tl, whhc, h_prev, c_prev, h_new[:])
                nc.scalar.copy(hc_all[:, t], h_new[:])  # f32 -> bf16 store
                h_prev = h_new

        # =================== phase G: generation LSTM (forward) ===========
        hg_prev_ap = None      # AP of h_{t-1} (lives inside hg segment tiles)
        h0g = stp.tile([128, 2, bl], F32, tag="h")
        c_prev = stp.tile([128, 2, bl], F32, tag="c")
        nc.vector.memset(h0g[:], 0.0)
        nc.vector.memset(c_prev[:], 0.0)
        hg_prev_ap = h0g[:]

        for seg in range(nseg):
            t0 = seg * tseg
            xg = xinp.tile([128, tseg, bl], F32, tag="xc0")
            nc.sync.dma_start(xg[:], d_xgT.ap()[0:128, t0:t0 + tseg])
            xp = xpp.tile([128, tseg, NQ, bl], F32, tag="xp")
            for q in range(NQ):
                ps = psb.tile([128, tseg, bl], F32, tag="psb")
                nc.tensor.matmul(ps[:], wgx[:, 128 * q:128 * (q + 1)],
                                 xg[:], start=True, stop=False)
                for k in range(2):
                    nc.tensor.matmul(ps[:], wghc[k][:, 128 * q:128 * (q + 1)],
                                     hc_all[:, t0:t0 + tseg, k, :],
                                     start=False, stop=(k == 1))
                nc.scalar.activation(xp[:, :, q, :], ps[:], AF.Identity,
                                     bias=bg_sb[:, q:q + 1])
            hg_seg = hgp.tile([128, tseg, 2, bl], F32, tag="hg")
            for tl in range(tseg):
                c_prev = scan_step(xp, tl, whhg, hg_prev_ap, c_prev,
                                   hg_seg[:, tl])
                hg_prev_ap = hg_seg[:, tl]
            # ---- MLP head for this segment ----
            ps1 = psb.tile([128, tseg, bl], F32, tag="psb")
            for k in range(2):
                nc.tensor.matmul(ps1[:], w1t[k][:], hg_seg[:, :, k, :],
                                 start=(k == 0), stop=(k == 1))
            y1 = yp.tile([128, tseg, bl], F32, tag="y1")
            nc.scalar.activation(y1[:], ps1[:], AF.Relu, bias=b1_sb[:, 0:1])
            ps2 = psb.tile([128, tseg, bl], F32, tag="psb")
            nc.tensor.matmul(ps2[:], w2t[:], y1[:], start=True, stop=True)
            y2 = yp.tile([128, tseg, bl], F32, tag="y2")
            nc.scalar.activation(y2[:], ps2[:], AF.Identity, bias=b2_sb[:, 0:1])
            nc.sync.dma_start(d_out.ap()[:, t0:t0 + tseg], y2[:])

    nc.compile()
    return nc, "out"


_PROGRAM_CACHE = {}


def get_program(s=S_FULL, tseg=TSEG, bl=BL):
    key = (s, tseg, bl)
    if key not in _PROGRAM_CACHE:
        _PROGRAM_CACHE[key] = build_program(s, tseg, bl)
    return _PROGRAM_CACHE[key]


# --------------------------------------------------------------------------
# entry point
# --------------------------------------------------------------------------

def kernel(**inputs) -> np.ndarray:
    s, b = np.asarray(inputs["seq"]).shape[:2]
    assert (s, b) == (S_FULL, B_FULL)
    nc, out_name = get_program()
    w = prep_weights(inputs)
    in_maps = []
    for core in range(NCORES):
        c0 = core * BL
        m = dict(w)
        m.update(prep_core_inputs(inputs["seq"], inputs["seq_constraints"],
                                  c0, c0 + BL, S_FULL))
        in_maps.append(m)
    res = run_bass_kernel_spmd(nc, in_maps, core_ids=list(range(NCORES)))
    # per-core out: [F, S, BL] -> [S, BL, F]; concat cores along batch
    parts = [np.transpose(res.results[c][out_name], (1, 2, 0))
             for c in range(NCORES)]
    return np.ascontiguousarray(np.concatenate(parts, axis=1))
